# revision 22
# baseline (speedup 1.0000x reference)
"""ComplexAttention Trainium2 kernel (8 NeuronCores, SPMD).

Math: the reference "complex" attention reduces exactly to standard
single-head attention with head_dim 2D=2048 and scale 1/sqrt(D):
  Re(q . conj(k)) over interleaved (r,i) pairs == full dot product q.k
  interleave(o_r, o_i)                         == softmax_w @ v_full

Further algebraic fusion (host-side, weights only):
  logits[s,t] = hs[s] @ A @ hs[t]^T + (hs @ u2)[t]   (+ per-row const, dropped)
      A  = Wq^T Wk / sqrt(D),  u2 = Wk^T bq / sqrt(D)
  out[s]  = (P_un @ (hs @ MT))[s] / sumexp[s] + biasf
      MT = Wv^T Wo^T,  biasf = bo + Wo bv
so no explicit q/k/v projections are needed: the key matrix is hs itself.
The termt[t] bias folds into qhat: scores+termt = hsT^T (qhat + u2).

All matmul operands are fp16 (full PE rate, fp32 PSUM accumulate); the
host pre-transposes hs into the [p, dc, s] device layout and pre-packs
A / MT slabs so the device does no transposes.

Sharding: 8 cores = 4 batches x 2 query-halves. Each core gets its batch's
hidden_states rotated so its 1024 query rows are rows 0:1024; keys/values
span the full (rotated) sequence -- softmax over keys is permutation
invariant, so rotation is exact.
"""

import math
import os
import time

import numpy as np

B, S, D = 4, 2048, 1024
P = 128
NCORES = 8
SQ = S // 2          # query rows per core
DC = D // P          # 8  d-chunks
TT = S // P          # 16 t-tiles
ST = SQ // P         # 8  s-tiles
FQ = 2               # f output slabs
FW = D // FQ         # 512
SC = 4               # hsT DMA chunks along s

_CACHE = {}
LAST_TIMING = {}


def _mm_np_dtype():
    return (np.float16
            if os.environ.get("CPLX_MM_DTYPE", "f16") == "f16"
            else np.float32)


def _emit(nc, tc, tile, mybir, aps):
    import concourse.bass as bass
    f32 = mybir.dt.float32
    mdt = (mybir.dt.float16
           if os.environ.get("CPLX_MM_DTYPE", "f16") == "f16"
           else mybir.dt.bfloat16)
    Exp = mybir.ActivationFunctionType.Exp
    Copy = mybir.ActivationFunctionType.Copy

    xT, Aq, MTq, u2s, onesc, biasb, y = (
        aps["xT"], aps["Aq"], aps["MTq"], aps["u2s"], aps["onesc"],
        aps["biasb"], aps["y"],
    )
    CS = S // SC
    xT3 = xT.rearrange("p (c r) -> p c r", c=SC)          # [P, SC, DC*CS]

    TH = TT // 2         # own t-tiles (local queries' rows)
    with (
        tc.tile_pool(name="persist", bufs=1) as persist,
        tc.tile_pool(name="psum_mm", bufs=6, space="PSUM") as psum_mm,
        tc.tile_pool(name="dram", bufs=1, space="DRAM") as dram,
    ):
        hsT = persist.tile([P, SC, DC, CS], mdt)       # 32 KB/p, s-chunk major
        Afull = persist.tile([P, DC, DC * P], mdt)     # 16 KB/p
        qhat2 = persist.tile([P, DC, SQ], mdt)         # 16 KB/p
        expT = persist.tile([P, TT, SQ], mdt)          # 32 KB/p
        vWoB = persist.tile([P, FQ, TT, FW], mdt)      # 32 KB/p
        mfull = persist.tile([P, FQ, DC, FW], mdt)     # 16 KB/p
        recipS = persist.tile([P, ST], f32)            # striped 1/sumexp
        u2s_sb = persist.tile([P, DC], f32)
        onesc_sb = persist.tile([P, 2], mdt)
        biasb_sb = persist.tile([P, D], f32)           # 4 KB/p

        # vWo pair exchange buffers (own local-t half out, both halves back)
        HB = FQ * TH * FW                              # elems/partition per block
        vloc = dram.tile([P, HB], mdt, name="vloc")
        vgath = dram.tile([2 * P, HB], mdt, name="vgath")

        def hsl(dc, t0, w):
            # hsT AP for s/t range [t0, t0+w) within one s-chunk
            c, r = divmod(t0, CS)
            return hsT[:, c, dc, r:r + w]

        # ---- input DMA, first-needed first; all per-partition contiguous ----
        nc.sync.dma_start(Afull[:, 0, :],
                          Aq[:, :DC * P].rearrange("p (o n) -> p o n", o=DC))
        # chunks 0/1 split by dc-half: P1's chain consumes slabs in dc
        # order, so finer completion grains let the PE start ~3us earlier
        HD = DC // 2
        for sc in range(2):
            for h in range(2):
                nc.sync.dma_start(
                    hsT[:, sc, h * HD:(h + 1) * HD, :],
                    xT3[:, sc, h * HD * CS:(h + 1) * HD * CS].rearrange(
                        "p (o j) -> p o j", o=HD))
            if sc == 0:
                nc.sync.dma_start(u2s_sb, u2s)
        for dpt in range(1, DC):
            nc.sync.dma_start(
                Afull[:, dpt, :],
                Aq[:, dpt * DC * P:(dpt + 1) * DC * P].rearrange(
                    "p (o n) -> p o n", o=DC))
        for q in range(FQ):
            nc.sync.dma_start(
                mfull[:, q],
                MTq[:, q * DC * FW:(q + 1) * DC * FW].rearrange(
                    "p (o f) -> p o f", o=DC))
        for sc in range(2, SC):
            nc.sync.dma_start(
                hsT[:, sc], xT3[:, sc, :].rearrange("p (o j) -> p o j", o=DC))
        nc.sync.dma_start(onesc_sb, onesc)
        nc.sync.dma_start(biasb_sb, biasb)

        # ---- P1: qhat2[d', s] = sum_d A[d, d'] hsT[d, s(q)] + u2[d'] ----
        # sh outer: the whole first sweep reads only hsT chunk 0, giving
        # the chunk-1 DMA a 13us head start on a cold HBM stream.
        for sh in range(2):
            for dpt in range(DC):
                ps = psum_mm.tile([P, 512], f32, tag="mm", name="mm_ps")
                for dc in range(DC):
                    nc.tensor.matmul(
                        ps,
                        lhsT=Afull[:, dpt, dc * P:(dc + 1) * P],
                        rhs=hsT[:, sh, dc, :],
                        start=(dc == 0),
                        stop=(dc == DC - 1),
                    )
                nc.vector.tensor_scalar_add(
                    out=qhat2[:, dpt, sh * 512:(sh + 1) * 512],
                    in0=ps, scalar1=u2s_sb[:, dpt:dpt + 1])

        # ---- vWo for OWN local t-half (rows 0:1024), both f slabs ----
        for q in range(FQ):
            for tt in range(TH):
                vp = psum_mm.tile([P, 512], f32, tag="mm",
                                  name="mm_ps")[:, :FW]
                for dc in range(DC):
                    nc.tensor.matmul(
                        vp,
                        lhsT=hsl(dc, tt * P, P),
                        rhs=mfull[:, q, dc, :],
                        start=(dc == 0),
                        stop=(dc == DC - 1),
                    )
                nc.vector.tensor_copy(out=vWoB[:, q, tt, :], in_=vp)

        # ---- exchange: AllGather own half within the batch pair, then
        # pull the peer's block into local rows 1024:2048. Peer's local
        # rows 0:1024 are exactly this core's rows 1024:2048 (rotation). --
        nc.sync.dma_start(
            vloc.rearrange("p (q t f) -> p q t f", q=FQ, t=TH),
            vWoB[:, :, 0:TH, :])
        nc.gpsimd.collective_compute(
            "AllGather", mybir.AluOpType.bypass,
            replica_groups=[[2 * i, 2 * i + 1] for i in range(NCORES // 2)],
            ins=[vloc[:, :]], outs=[vgath[:, :]],
        )
        pid = nc.sync.partition_id()
        peer = 1 - pid % 2
        for q in range(FQ):
            for i in range(TH):
                nc.sync.dma_start(
                    vWoB[:, q, TH + i, :],
                    vgath[bass.ds(peer * P, P),
                          q * TH * FW + i * FW:q * TH * FW + (i + 1) * FW])

        # ---- P2: scoresT[t, s] -> exp ----
        for tt in range(TT):
            for sh in range(2):
                ps = psum_mm.tile([P, 512], f32, tag="mm", name="mm_ps")
                for dc in range(DC):
                    nc.tensor.matmul(
                        ps,
                        lhsT=hsl(dc, tt * P, P),
                        rhs=qhat2[:, dc, sh * 512:(sh + 1) * 512],
                        start=(dc == 0),
                        stop=(dc == DC - 1),
                    )
                nc.scalar.activation(
                    expT[:, tt, sh * 512:(sh + 1) * 512], ps, Exp)

        # ---- tail: G -> scale -> bias -> out; the striped sumexp chains
        # (recipS[p, st] = 1/sum_t expT[t, st*128+p]) interleave with the
        # G groups so the PE stream stays dense (no HAM clock demotion) --
        with tc.tile_pool(name="outp", bufs=3) as outp:
            for q in range(FQ):
                for st in range(ST):
                    if q == 0:
                        sp = psum_mm.tile([P, 512], f32, tag="mm",
                                          name="mm_ps")[:, :2]
                        for tt in range(TT):
                            nc.tensor.matmul(
                                sp,
                                lhsT=expT[:, tt, st * P:(st + 1) * P],
                                rhs=onesc_sb,
                                start=(tt == 0),
                                stop=(tt == TT - 1),
                            )
                        nc.vector.reciprocal(recipS[:, st:st + 1], sp[:, 0:1])
                    gp = psum_mm.tile([P, 512], f32, tag="mm",
                                      name="mm_ps")[:, :FW]
                    for tt in range(TT):
                        nc.tensor.matmul(
                            gp,
                            lhsT=expT[:, tt, st * P:(st + 1) * P],
                            rhs=vWoB[:, q, tt, :],
                            start=(tt == 0),
                            stop=(tt == TT - 1),
                        )
                    ot = outp.tile([P, FW], f32, tag="ot", name="ot")
                    nc.scalar.activation(
                        ot, gp, Copy, scale=recipS[:, st:st + 1])
                    nc.vector.tensor_add(
                        out=ot, in0=ot,
                        in1=biasb_sb[:, q * FW:(q + 1) * FW])
                    nc.sync.dma_start(
                        y[q * SQ + st * P:q * SQ + (st + 1) * P, :], ot)


def _build():
    key = ("nc", os.environ.get("CPLX_MM_DTYPE", "f16"))
    if key in _CACHE:
        return _CACHE[key]
    import concourse.bass as bass  # noqa: F401
    import concourse.tile as tile
    import concourse.mybir as mybir
    from concourse import bacc

    f32 = mybir.dt.float32
    mdt = (mybir.dt.float16
           if os.environ.get("CPLX_MM_DTYPE", "f16") == "f16"
           else mybir.dt.bfloat16)
    nc = bacc.Bacc("TRN2", target_bir_lowering=False, debug=False,
                   enable_asserts=False, num_devices=NCORES)
    aps = {
        "xT": nc.dram_tensor("xT", [P, DC * S], mdt,
                             kind="ExternalInput").ap(),
        "Aq": nc.dram_tensor("Aq", [P, DC * DC * P], mdt,
                             kind="ExternalInput").ap(),
        "MTq": nc.dram_tensor("MTq", [P, FQ * DC * FW], mdt,
                              kind="ExternalInput").ap(),
        "u2s": nc.dram_tensor("u2s", [P, DC], f32, kind="ExternalInput").ap(),
        "onesc": nc.dram_tensor("onesc", [P, 2], mdt,
                                kind="ExternalInput").ap(),
        "biasb": nc.dram_tensor("biasb", [P, D], f32,
                                kind="ExternalInput").ap(),
        "y": nc.dram_tensor("y", [FQ * SQ, FW], f32,
                            kind="ExternalOutput").ap(),
    }
    with tile.TileContext(nc) as tc:
        _emit(nc, tc, tile, mybir, aps)
    nc.compile()
    _CACHE[key] = nc
    return nc


def _host_prep(inputs):
    hs = np.asarray(inputs["hidden_states"], dtype=np.float32)
    Wq = np.asarray(inputs["Wq"], dtype=np.float64)
    bq = np.asarray(inputs["bq"], dtype=np.float64)
    Wk = np.asarray(inputs["Wk"], dtype=np.float64)
    Wv = np.asarray(inputs["Wv"], dtype=np.float64)
    bv = np.asarray(inputs["bv"], dtype=np.float64)
    Wo = np.asarray(inputs["Wo"], dtype=np.float64)
    bo = np.asarray(inputs["bo"], dtype=np.float64)
    mdt = _mm_np_dtype()

    scale = 1.0 / math.sqrt(D)
    A = ((Wq.T @ Wk) * scale).astype(np.float32)            # [d, d']
    u2 = ((Wk.T @ bq) * scale).astype(np.float32)           # [d']
    MT = (Wv.T @ Wo.T).astype(np.float32)                   # [d, f]
    biasf = (bo + Wo @ bv).astype(np.float32)               # [f]

    # Aq[p, dpt, o, n] = A[o*128+p, dpt*128+n]  -> [128, DC*DC*P]
    Aq = np.ascontiguousarray(
        A.reshape(DC, P, DC, P).transpose(1, 2, 0, 3).reshape(P, -1)
    ).astype(mdt)
    # MTq[p, q, o, f] = MT[o*128+p, q*FW+f]  -> [128, FQ*DC*FW]
    MTq = np.ascontiguousarray(
        MT.reshape(DC, P, FQ, FW).transpose(1, 2, 0, 3).reshape(P, -1)
    ).astype(mdt)
    u2s = np.ascontiguousarray(u2.reshape(DC, P).T)         # [128, 8] striped
    onesc = np.ones((P, 2), dtype=mdt)
    biasb = np.ascontiguousarray(
        np.broadcast_to(biasf[None, :], (P, D)))            # [128, 1024]

    in_maps = []
    for core in range(NCORES):
        b, half = core // 2, core % 2
        if half == 0:
            xc = hs[b]
        else:
            xc = np.concatenate([hs[b, SQ:], hs[b, :SQ]], axis=0)
        # xT[p, sc, dc, j] = xc[sc*CS+j, dc*128+p]  -> [128, SC*DC*CS]
        CS = S // SC
        xT = np.ascontiguousarray(
            xc.reshape(SC, CS, DC, P).transpose(3, 0, 2, 1).reshape(P, -1)
        ).astype(mdt)
        in_maps.append({
            "xT": xT,
            "Aq": Aq,
            "MTq": MTq,
            "u2s": u2s,
            "onesc": onesc,
            "biasb": biasb,
        })
    return in_maps


def _make_runner(nc, in_maps):
    """Persistent jitted SPMD runner (mirrors bass2jax.run_bass_via_pjrt)."""
    import jax
    import numpy as np
    from jax.experimental.shard_map import shard_map
    from jax.sharding import Mesh, PartitionSpec
    import concourse.mybir as mybir
    from concourse import bass2jax

    bass2jax.install_neuronx_cc_hook()
    partition_name = (
        nc.partition_id_tensor.name if nc.partition_id_tensor else None)

    in_names, out_names, out_avals, zero_outs = [], [], [], []
    for alloc in nc.m.functions[0].allocations:
        if not isinstance(alloc, mybir.MemoryLocationSet):
            continue
        name = alloc.memorylocations[0].name
        if alloc.kind == "ExternalInput":
            if name != partition_name:
                in_names.append(name)
        elif alloc.kind == "ExternalOutput":
            np_dt = mybir.dt.np(alloc.dtype)
            out_names.append(name)
            out_avals.append(
                jax.core.ShapedArray(tuple(alloc.tensor_shape), np_dt))
            zero_outs.append(
                np.zeros(tuple(alloc.tensor_shape), np_dt))

    n_params = len(in_names)
    n_outs = len(out_avals)
    all_in_names = in_names + out_names
    if partition_name is not None:
        all_in_names = all_in_names + [partition_name]

    def _body(*args):
        operands = list(args)
        if partition_name is not None:
            operands.append(bass2jax.partition_id_tensor())
        outs = bass2jax._bass_exec_p.bind(
            *operands,
            out_avals=tuple(out_avals),
            in_names=tuple(all_in_names),
            out_names=tuple(out_names),
            lowering_input_output_aliases=(),
            sim_require_finite=True,
            sim_require_nnan=True,
            nc=nc,
        )
        return tuple(outs)

    devices = jax.devices()[:NCORES]
    mesh = Mesh(np.asarray(devices), ("core",))
    in_specs = (PartitionSpec("core"),) * (n_params + n_outs)
    out_specs = (PartitionSpec("core"),) * n_outs
    sharded = jax.jit(
        shard_map(_body, mesh=mesh, in_specs=in_specs, out_specs=out_specs,
                  check_rep=False),
        keep_unused=True,
    )

    concat_in = [
        np.concatenate([in_maps[c][nm] for c in range(NCORES)], axis=0)
        for nm in in_names
    ]
    concat_zeros = [
        np.zeros((NCORES * z.shape[0], *z.shape[1:]), z.dtype)
        for z in zero_outs
    ]
    from jax.sharding import NamedSharding
    sharding = NamedSharding(mesh, PartitionSpec("core"))
    args = [jax.device_put(a, sharding)
            for a in [*concat_in, *concat_zeros]]
    jax.block_until_ready(args)

    def run():
        out = sharded(*args)
        jax.block_until_ready(out)
        return out

    def run_queued(n):
        # queue n executions back-to-back, block once: amortizes the
        # multi-ms axon dispatch overhead so the marginal cost per exec
        # approaches true device time
        o = None
        t0 = time.perf_counter()
        for _ in range(n):
            o = sharded(*args)
        jax.block_until_ready(o)
        return time.perf_counter() - t0

    run.queued = run_queued
    return run, out_names, out_avals


def kernel(**inputs):
    in_maps = _host_prep(inputs)
    nc = _build()
    run, out_names, out_avals = _make_runner(nc, in_maps)

    t0 = time.perf_counter()
    out_arrs = run()  # first call compiles
    t1 = time.perf_counter()

    n_timed = int(os.environ.get("CPLX_TIMED_ITERS", "0"))
    times = []
    for _ in range(n_timed):
        ts = time.perf_counter()
        run()
        times.append(time.perf_counter() - ts)
    marginal = None
    if n_timed:
        # slope over queued batches: subtracts fixed dispatch overhead
        lo, hi = 8, 128
        t_lo = min(run.queued(lo) for _ in range(2))
        t_hi = min(run.queued(hi) for _ in range(2))
        marginal = (t_hi - t_lo) / (hi - lo)
    LAST_TIMING.clear()
    LAST_TIMING.update({
        "first_call_s": t1 - t0,
        "timed_iters_s": times,
        "best_iter_s": min(times) if times else None,
        "marginal_exec_s": marginal,
    })

    yi = out_names.index("y")
    ys = np.asarray(out_arrs[yi]).reshape(NCORES, FQ, SQ, FW)

    out = np.empty((B, S, D), dtype=np.float32)
    for core in range(NCORES):
        b, half = core // 2, core % 2
        for q in range(FQ):
            out[b, half * SQ:(half + 1) * SQ, q * FW:(q + 1) * FW] = (
                ys[core, q])
    return out
